# revision 36
# baseline (speedup 1.0000x reference)
"""Bass/Trainium2 kernel v5 for nn_NormAttention (causal linear attention).

Batch+head-sharded SPMD across 8 NeuronCores, no collectives:
core c owns batch b = c//4 and heads {2*(c%4), 2*(c%4)+1} (2048 rows,
2 heads per core: half the x traffic / half the output partial of the
v2 head-only split, and V-proj / Wo run with the full 128-wide array).

Execution shape (per core, rows chunked at C=128, 16 chunks):
  - 4 projection chunks of 512 rows; per chunk 12 N=512 matmuls
    (q|k per head, packed v) — big streaming work for the PE.
  - per row-chunk: 2 score matmuls (N=128, K=64), masked on DVE; po =
    V_rows.T @ at + S_prefix.T @ q (4 matmuls N=128); states K.T@V~
    (2 matmuls N=64); Wo [128,512] (N=512, K=128 both heads packed).
  - row-major K/V tiles come from 6 batched xbar DMA-transposes
    ([128, 512-1024] each; the ~1us/op fixed cost made 16 small ones a
    serial bottleneck in v3).
  - scores and states+prefix run 2 steps ahead of po so the
    states->DVE copy->Pool add->PE inter-matmul cross-engine chain is
    never on the critical path.
  - 2 dead N=512 "filler" matmuls per step keep the PE HAM activity
    window above the clock-gate threshold (otherwise the attention
    phase runs at 1.2GHz instead of 2.4GHz).
  - DMA queues: sync = k shifts + transposes; scalar = x0/x1 loads;
    gpsimd = const blob + x2/x3 + paired output stores.

1/|q| and 1/|k| span all 8 heads (they live on different cores), so
they are computed host-side (as in the graded v2 baseline); 1/|k| is
folded into V on device, 1/|q| + bo into the host-side unshard sum of
the 4 per-batch partials.
"""
import numpy as np
import ml_dtypes

import concourse.bacc as bacc
import concourse.tile as tile
import concourse.mybir as mybir
import concourse.bass_utils as bass_utils

F32 = mybir.dt.float32
BF16 = mybir.dt.bfloat16
BF = ml_dtypes.bfloat16
AF = mybir.ActivationFunctionType
ALU = mybir.AluOpType

B, L, E, H, HD = 2, 2048, 512, 8, 64
N = B * L
NCORES = 8
RPC = 2048              # rows per core (one batch)
KT = 4                  # contraction k-tiles (E // 128)
C = 128                 # attention row-chunk
NCH = RPC // C          # 16 row chunks per core
PCW = 512               # proj chunk width (rows)
NPC = RPC // PCW        # 4 proj chunks
EPS = 1e-12

# const blob column offsets (bf16): wqk | wv | wo2 | mask2 | rk2
OW_QK, OW_V, OW_O, OW_M, OW_R = 0, 1024, 1536, 2048, 2304
CBLOB_W = 2304 + RPC

_cache = {}


def _build():
    nc = bacc.Bacc("TRN2", target_bir_lowering=False, debug=False,
                   num_devices=NCORES)

    xt_d = nc.dram_tensor("xt", [128, NPC, KT, PCW], BF16,
                          kind="ExternalInput").ap()
    cb_d = nc.dram_tensor("cb", [128, CBLOB_W], BF16,
                          kind="ExternalInput").ap()
    bias_d = nc.dram_tensor("bias", [128, 3], F32,
                            kind="ExternalInput").ap()
    out_d = nc.dram_tensor("out", [RPC, E], BF16, kind="ExternalOutput").ap()

    with tile.TileContext(nc) as tc:
        with (
            tc.tile_pool(name="const", bufs=1) as const,
            tc.tile_pool(name="bigp", bufs=1) as bigp,
            tc.tile_pool(name="xtp", bufs=4) as xtp,
            tc.tile_pool(name="atp", bufs=4) as atp,
            tc.tile_pool(name="otp", bufs=4) as otp,
            tc.tile_pool(name="ssbp", bufs=3) as ssbp,
            tc.tile_pool(name="osbp", bufs=3) as osbp,
            tc.tile_pool(name="pjqk", bufs=2, space="PSUM") as pjqk,
            tc.tile_pool(name="pjv", bufs=1, space="PSUM") as pjv,
            tc.tile_pool(name="ps2", bufs=1, space="PSUM") as ps2,
            tc.tile_pool(name="ppo", bufs=1, space="PSUM") as ppo,
            tc.tile_pool(name="pstp", bufs=1, space="PSUM") as pstp,
            tc.tile_pool(name="pwo", bufs=2, space="PSUM") as pwo,
        ):
            # ---- constants: wqk+wv first (proj(0) deps), then the rest
            # (wo2/mask2/rk2 after x3 — they are needed later than x3)
            cblob = const.tile([128, CBLOB_W], BF16)
            bias_sb = const.tile([128, 3], F32)
            nc.gpsimd.dma_start(cblob[:, 0:OW_O], cb_d[:, 0:OW_O])
            nc.gpsimd.dma_start(bias_sb[:], bias_d)

            def w_qk(k, h):
                o = OW_QK + k * 256 + h * 128
                return cblob[:, o:o + 128]

            def w_v(k):
                o = OW_V + k * 128
                return cblob[:, o:o + 128]

            wo2_sb = cblob[:, OW_O:OW_O + E]
            mask2_sb = cblob[:, OW_M:OW_M + 256].rearrange(
                "p (h c) -> p h c", h=2)
            rk2_sb = cblob[:, OW_R:OW_R + RPC]
            bqk_sb = bias_sb[:, 0:2]
            bvp_sb = bias_sb[:, 2:3]

            # ---- x chunk loads: 0,1 on scalar queue, 2,3 on gpsimd -------
            xtiles = {}
            for pc in range(NPC):
                xtile = xtp.tile([128, KT, PCW], BF16, tag="xt", name="xtile")
                eng = nc.scalar if pc < 3 else nc.gpsimd
                eng.dma_start(xtile[:], xt_d[:, pc, :, :])
                xtiles[pc] = xtile
            nc.gpsimd.dma_start(cblob[:, OW_O:], cb_d[:, OW_O:])

            # ---- PE warm-up bridge while the first DMAs land -------------
            wsc = const.tile([128, 512], BF16)
            nc.vector.memset(wsc[:], 0.0)
            warm = pwo.tile([128, 512], F32, tag="wps", name="warm")
            NWARM = 10
            for i in range(NWARM):
                nc.tensor.matmul(warm[:], wsc[:, 0:128], wsc[:],
                                 start=(i == 0), stop=(i == NWARM - 1))

            # ---- persistent activations ----------------------------------
            # ktv0 = [k_h0 0:64 (DMA shift, also scores lhsT) | v~_h0 64:]
            # ktv1 = [v~_h1 0:64 | k_h1 64:128 (plain copy)]  (vp packed
            #   [v_h1; v_h0] so both stt writes are partition-aligned)
            # kc1  = k_h1 shifted to partitions 0:64 (scores lhsT for h1)
            # every proj-phase tensor is a separate tile PER PROJ CHUNK so
            # all tile dependencies are exact (whole-tile) — a reader of
            # pc's data can never be conservatively chained to a later
            # pc's DMA write
            qkt = [[bigp.tile([128, PCW], BF16, name=f"qkt{h}_{pc}")
                    for pc in range(NPC)] for h in range(2)]
            ktv0 = [bigp.tile([128, PCW], BF16, name=f"ktv0_{pc}")
                    for pc in range(NPC)]
            ktv1 = [bigp.tile([128, PCW], BF16, name=f"ktv1_{pc}")
                    for pc in range(NPC)]
            kc1 = [bigp.tile([64, PCW], BF16, name=f"kc1_{pc}")
                   for pc in range(NPC)]
            # kvr[0] cols = [k 0:64 | v 64:128]; kvr[1] = [v 0:64 | k 64:]
            kvr = [[bigp.tile([128, 4, 2 * HD], BF16, name=f"kvr{h}_{pc}")
                    for pc in range(NPC)] for h in range(2)]

            def kvr_at(h, cl):
                return kvr[h][cl // 4], cl % 4

            ps2t = ps2.tile([128, 4, C], F32, tag="s2", name="s2")
            ppo_t = ppo.tile([128, 4, C], F32, tag="po", name="po")
            pst_t = pstp.tile([HD, 8, HD], F32, tag="st", name="st")
            sp_of = {}          # cl -> [64, 2(head), 64] bf16 state product
            pref = {}           # cl -> [64, 2(head), 64] bf16 S_{<cl}

            def proj(pc):
                xtile = xtiles[pc]
                sl = slice(pc * PCW, (pc + 1) * PCW)
                for h in (0, 1):
                    qk = pjqk.tile([128, PCW], F32, tag="pj", name="qkps")
                    for k in range(KT):
                        nc.tensor.matmul(qk[:], w_qk(k, h), xtile[:, k, :],
                                         start=(k == 0), stop=(k == KT - 1))
                    nc.scalar.activation(qkt[h][pc][:], qk[:], AF.Relu,
                                         bias=bqk_sb[:, h:h + 1])
                # k shifts on sync (scores-critical, FIFO before transposes)
                nc.sync.dma_start(ktv0[pc][0:64, :], qkt[0][pc][64:128, :])
                nc.sync.dma_start(kc1[pc][:], qkt[1][pc][64:128, :])
                # plain-aligned k_h1 copy for the transpose input
                nc.sync.dma_start(ktv1[pc][64:128, :], qkt[1][pc][64:128, :])
                # v psum double-buffers by borrowing the (idle until step 1)
                # pwo bank for odd pc — back-to-back projs don't serialize
                # on the DVE stt drain
                pool = pjv if pc % 2 == 0 else pwo
                vp = pool.tile([128, PCW], F32,
                               tag="pjv" if pc % 2 == 0 else "wps",
                               name="vps")
                for k in range(KT):
                    nc.tensor.matmul(vp[:], w_v(k), xtile[:, k, :],
                                     start=(k == 0), stop=(k == KT - 1))
                # v~ = (v + bv) * (1/|k_row|); vp = [v_h1; v_h0] so both
                # halves land partition-aligned
                nc.vector.scalar_tensor_tensor(
                    ktv1[pc][0:64, :], vp[0:64, :], bvp_sb[0:64, :],
                    rk2_sb[0:64, sl], op0=ALU.add, op1=ALU.mult)
                nc.vector.scalar_tensor_tensor(
                    ktv0[pc][64:128, :], vp[64:128, :], bvp_sb[64:128, :],
                    rk2_sb[64:128, sl], op0=ALU.add, op1=ALU.mult)
                # row-major kvr for this pc's 4 chunks (xbar transposes)
                nc.sync.dma_start_transpose(kvr[0][pc][:], ktv0[pc][:])
                nc.sync.dma_start_transpose(kvr[1][pc][:], ktv1[pc][:])

            # per-head column slices of row-major kvr: [k | v] vs [v | k]
            KSL = (slice(0, HD), slice(HD, 2 * HD))
            VSL = (slice(HD, 2 * HD), slice(0, HD))

            def prework(cl):
                # scores for both heads of chunk cl + causal mask (DVE)
                pc, lo = cl // 4, (cl % 4) * C
                rows = slice(lo, lo + C)
                s0 = (cl % 2) * 2
                nc.tensor.matmul(ps2t[:, s0, :], ktv0[pc][0:64, rows],
                                 qkt[0][pc][0:64, rows],
                                 start=True, stop=True)
                nc.tensor.matmul(ps2t[:, s0 + 1, :], kc1[pc][:, rows],
                                 qkt[1][pc][0:64, rows],
                                 start=True, stop=True)
                at2 = atp.tile([128, 2, C], BF16, name="at2")
                nc.vector.tensor_mul(at2[:], ps2t[:, s0:s0 + 2, :],
                                     mask2_sb[:])
                return at2

            def states_block(cl):
                s = (2 * cl) % 8
                for h in (0, 1):
                    kt, j = kvr_at(h, cl)
                    nc.tensor.matmul(pst_t[:, s + h, :],
                                     kt[:, j, KSL[h]],
                                     kt[:, j, VSL[h]],
                                     start=True, stop=True)
                nxt = cl + 1
                if nxt >= NCH:
                    return
                # prefix fused with the PSUM drain: one DVE op per chunk
                # (pref[cl+1] = pref[cl] + S_cl), no Pool / no extra copy
                pf = ssbp.tile([HD, 2, HD], BF16, tag="pref", bufs=8,
                               name="pref")
                if cl == 0:
                    nc.vector.tensor_copy(pf[:], pst_t[:, s:s + 2, :])
                else:
                    nc.vector.tensor_add(pf[:], pref[cl][:],
                                         pst_t[:, s:s + 2, :])
                pref[nxt] = pf

            def po_block(cl, at2):
                pc, lo = cl // 4, (cl % 4) * C
                rows = slice(lo, lo + C)
                s = cl % 4
                for h in (0, 1):
                    kt, j = kvr_at(h, cl)
                    nc.tensor.matmul(ppo_t[h * HD:(h + 1) * HD, s, :],
                                     kt[:, j, VSL[h]], at2[:, h, :],
                                     start=True, stop=(cl == 0))
                    if cl > 0:
                        nc.tensor.matmul(ppo_t[h * HD:(h + 1) * HD, s, :],
                                         pref[cl][:, h, :],
                                         qkt[h][pc][0:64, rows],
                                         start=False, stop=True)
                # po -> SBUF bf16 (ACT; DVE is loaded with mask+states)
                ot = otp.tile([128, C], BF16, name="ot")
                nc.scalar.copy(ot[:], ppo_t[:, s, :])
                return ot

            ob_cur = {}

            def wo_block(cl, ot):
                pw = pwo.tile([128, E], F32, tag="wps", name="wps")
                nc.tensor.matmul(pw[:], ot[:], wo2_sb[:],
                                 start=True, stop=True)
                if cl % 2 == 0:
                    ob_cur["t"] = osbp.tile([128, 2, E], BF16, tag="osb",
                                            name="osb")
                ob = ob_cur["t"]
                j = cl % 2
                # uneven column split: ACT is lighter-loaded than DVE
                nc.scalar.copy(ob[:, j, 0:384], pw[:, 0:384])
                nc.vector.tensor_copy(ob[:, j, 384:E], pw[:, 384:E])
                if j == 1:
                    dst = out_d[(cl - 1) * C:(cl + 1) * C, :].rearrange(
                        "(j p) e -> p j e", j=2)
                    nc.gpsimd.dma_start(dst, ob[:])

            def filler(n):
                # dead N=512 matmuls to keep the PE HAM activity monitor
                # above its throttle threshold (else the clock gate halves
                # the PE clock for the small-matmul attention phase)
                fw = pwo.tile([128, 512], F32, tag="wps", name="fill")
                for i in range(n):
                    nc.tensor.matmul(fw[:], wsc[:, 0:128], wsc[:],
                                     start=(i == 0), stop=(i == n - 1))

            # ---- pipeline ------------------------------------------------
            # Projections are front-loaded (0-2 before the loop, 3 at step
            # 1): the PE streams ~14us of dense N=512 matmuls while the
            # sync-queue shift->transpose convoy resolves, so every
            # row-major kvr tile is ready long before states needs it.
            # step cl: scores(cl+2) | po(cl) | Wo(cl-1) | states(cl+2);
            # the 2-step lookahead keeps the cross-engine consumers (DVE
            # mask, fused prefix-add) off the in-order PE queue's critical
            # path.
            proj(0)
            proj(1)
            at_of = {0: prework(0), 1: prework(1)}
            states_block(0)
            states_block(1)
            ot_of = {}
            for cl in range(NCH + 1):
                if cl == 0:
                    proj(2)
                elif cl == 2:
                    proj(3)
                elif cl == 4:
                    # contiguous ~3.4us matmul burst: if the proj(3)
                    # shift/transpose convoy stalled the PE and the HAM
                    # gate dropped to 1.2GHz, this flips it back to 2.4GHz
                    # for the remaining ~60% of the kernel
                    filler(8)
                elif cl < NCH:
                    filler(2 if cl > 4 else 1)
                if cl + 2 <= NCH - 1:
                    at_of[cl + 2] = prework(cl + 2)
                if cl <= NCH - 1:
                    ot_of[cl] = po_block(cl, at_of.pop(cl))
                if cl - 1 >= 0:
                    wo_block(cl - 1, ot_of.pop(cl - 1))
                if cl + 2 <= NCH - 1:
                    states_block(cl + 2)

    nc.compile()
    return nc


def _get_nc():
    if "nc" not in _cache:
        _cache["nc"] = _build()
    return _cache["nc"]


def _host_norms(xs, W, bias):
    """1/max(||relu(xs @ W.T + bias)||, eps) per row, flat [N] f32."""
    p = np.maximum(xs @ W.T + bias, 0.0)
    nrm = np.maximum(np.sqrt(np.sum(p * p, axis=1)), EPS)
    return (1.0 / nrm).astype(np.float32)


def kernel(query, Wq, bq, Wk, bk, Wv, bv, Wo, bo):
    query = np.asarray(query, dtype=np.float32)
    Wq, bq = np.asarray(Wq, np.float32), np.asarray(bq, np.float32)
    Wk, bk = np.asarray(Wk, np.float32), np.asarray(bk, np.float32)
    Wv, bv = np.asarray(Wv, np.float32), np.asarray(bv, np.float32)
    Wo, bo = np.asarray(Wo, np.float32), np.asarray(bo, np.float32)
    assert query.shape == (B, L, E)

    # x = query.reshape(L, B, E) (torch view), then b-major rows
    xs = np.ascontiguousarray(
        query.reshape(L, B, E).transpose(1, 0, 2)).reshape(N, E)

    rq = _host_norms(xs, Wq, bq)
    rk = _host_norms(xs, Wk, bk)

    # per-batch x tiles: [128, pc, kt, n'] with 4KB contiguous rows
    xt_b = []
    rk2_b = []
    for b in range(B):
        xb = xs[b * L:(b + 1) * L]
        xt_b.append(np.ascontiguousarray(
            xb.T.reshape(KT, 128, NPC, PCW).transpose(1, 2, 0, 3)).astype(BF))
        rk2_b.append(np.ascontiguousarray(np.broadcast_to(
            rk[b * L:(b + 1) * L][None, :], (128, RPC))).astype(BF))

    tri = np.triu(np.ones((C, C), np.float32)).astype(BF)
    mask2 = np.ascontiguousarray(
        np.broadcast_to(tri[:, None, :], (C, 2, C))).reshape(C, 2 * C)

    in_maps = []
    for c in range(NCORES):
        b = c // 4
        h0 = 2 * (c % 4)
        cols0 = slice(HD * h0, HD * (h0 + 1))
        cols1 = slice(HD * (h0 + 1), HD * (h0 + 2))
        wqk = np.empty((128, KT, 2, 128), np.float32)
        bqk = np.empty((128, 2), np.float32)
        for h, cols in enumerate((cols0, cols1)):
            wcat = np.concatenate([Wq[cols].T, Wk[cols].T], axis=1)
            wqk[:, :, h, :] = wcat.reshape(KT, 128, 128).transpose(1, 0, 2)
            bqk[:, h] = np.concatenate([bq[cols], bk[cols]])
        # vp psum layout is [v_h1 (0:64) | v_h0 (64:128)] — see ktv comments
        vcat = np.concatenate([Wv[cols1].T, Wv[cols0].T], axis=1)
        wv = vcat.reshape(KT, 128, 128).transpose(1, 0, 2)
        wo2 = np.concatenate([Wo[:, cols0].T, Wo[:, cols1].T], axis=0)
        cb = np.concatenate([
            wqk.reshape(128, KT * 256),
            wv.reshape(128, KT * 128),
            wo2,
            mask2,
            rk2_b[b],
        ], axis=1).astype(BF)
        assert cb.shape == (128, CBLOB_W)
        bias = np.concatenate(
            [bqk, np.concatenate([bv[cols1], bv[cols0]])[:, None]],
            axis=1).astype(np.float32)
        in_maps.append(dict(xt=xt_b[b], cb=cb, bias=bias))

    nc = _get_nc()
    res = bass_utils.run_bass_kernel_spmd(nc, in_maps,
                                          core_ids=list(range(NCORES)))
    total = np.zeros((N, E), np.float32)
    for c in range(NCORES):
        b = c // 4
        total[b * L:(b + 1) * L] += res.results[c]["out"].astype(np.float32)
    total *= rq[:, None]

    out = (total.reshape(B, L, E).transpose(1, 0, 2) + bo).reshape(B, L, E)
    return np.ascontiguousarray(out.astype(np.float32))


# revision 42
# speedup vs baseline: 1.0453x; 1.0453x over previous
"""Bass/Trainium2 kernel v5 for nn_NormAttention (causal linear attention).

Batch+head-sharded SPMD across 8 NeuronCores, no collectives:
core c owns batch b = c//4 and heads {2*(c%4), 2*(c%4)+1} (2048 rows,
2 heads per core: half the x traffic / half the output partial of the
v2 head-only split, and V-proj / Wo run with the full 128-wide array).

Execution shape (per core, rows chunked at C=128, 16 chunks):
  - 4 projection chunks of 512 rows; per chunk 12 N=512 matmuls
    (q|k per head, packed v) — big streaming work for the PE.
  - per row-chunk: 2 score matmuls (N=128, K=64), masked on DVE; po =
    V_rows.T @ at + S_prefix.T @ q (4 matmuls N=128); states K.T@V~
    (2 matmuls N=64); Wo [128,512] (N=512, K=128 both heads packed).
  - row-major K/V tiles come from 6 batched xbar DMA-transposes
    ([128, 512-1024] each; the ~1us/op fixed cost made 16 small ones a
    serial bottleneck in v3).
  - scores and states+prefix run 2 steps ahead of po so the
    states->DVE copy->Pool add->PE inter-matmul cross-engine chain is
    never on the critical path.
  - 2 dead N=512 "filler" matmuls per step keep the PE HAM activity
    window above the clock-gate threshold (otherwise the attention
    phase runs at 1.2GHz instead of 2.4GHz).
  - DMA queues: sync = k shifts + transposes; scalar = x0/x1 loads;
    gpsimd = const blob + x2/x3 + paired output stores.

1/|q| and 1/|k| span all 8 heads (they live on different cores), so
they are computed host-side (as in the graded v2 baseline); 1/|k| is
folded into V on device, 1/|q| + bo into the host-side unshard sum of
the 4 per-batch partials.
"""
import numpy as np
import ml_dtypes

import concourse.bacc as bacc
import concourse.tile as tile
import concourse.mybir as mybir
import concourse.bass_utils as bass_utils

F32 = mybir.dt.float32
BF16 = mybir.dt.bfloat16
BF = ml_dtypes.bfloat16
AF = mybir.ActivationFunctionType
ALU = mybir.AluOpType

B, L, E, H, HD = 2, 2048, 512, 8, 64
N = B * L
NCORES = 8
RPC = 2048              # rows per core (one batch)
KT = 4                  # contraction k-tiles (E // 128)
C = 128                 # attention row-chunk
NCH = RPC // C          # 16 row chunks per core
PCW = 512               # proj chunk width (rows)
NPC = RPC // PCW        # 4 proj chunks
EPS = 1e-12

# const blob column offsets (bf16): wqk | wv | wo2 | mask2 | rk2
OW_QK, OW_V, OW_O, OW_M, OW_R = 0, 1024, 1536, 2048, 2304
CBLOB_W = 2304 + RPC

_cache = {}


def _build():
    nc = bacc.Bacc("TRN2", target_bir_lowering=False, debug=False,
                   num_devices=NCORES)

    xt_d = nc.dram_tensor("xt", [128, NPC, KT, PCW], BF16,
                          kind="ExternalInput").ap()
    cb_d = nc.dram_tensor("cb", [128, CBLOB_W], BF16,
                          kind="ExternalInput").ap()
    bias_d = nc.dram_tensor("bias", [128, 3], F32,
                            kind="ExternalInput").ap()
    out_d = nc.dram_tensor("out", [RPC, E], BF16, kind="ExternalOutput").ap()

    with tile.TileContext(nc) as tc:
        with (
            tc.tile_pool(name="const", bufs=1) as const,
            tc.tile_pool(name="bigp", bufs=1) as bigp,
            tc.tile_pool(name="xtp", bufs=4) as xtp,
            tc.tile_pool(name="atp", bufs=4) as atp,
            tc.tile_pool(name="otp", bufs=4) as otp,
            tc.tile_pool(name="ssbp", bufs=3) as ssbp,
            tc.tile_pool(name="osbp", bufs=3) as osbp,
            tc.tile_pool(name="pjqk", bufs=2, space="PSUM") as pjqk,
            tc.tile_pool(name="pjv", bufs=1, space="PSUM") as pjv,
            tc.tile_pool(name="ps2", bufs=1, space="PSUM") as ps2,
            tc.tile_pool(name="ppo", bufs=1, space="PSUM") as ppo,
            tc.tile_pool(name="pstp", bufs=1, space="PSUM") as pstp,
            tc.tile_pool(name="pwo", bufs=2, space="PSUM") as pwo,
        ):
            # ---- constants: wqk+wv first (proj(0) deps), then the rest
            # (wo2/mask2/rk2 after x3 — they are needed later than x3)
            cblob = const.tile([128, CBLOB_W], BF16)
            bias_sb = const.tile([128, 3], F32)
            nc.gpsimd.dma_start(cblob[:, 0:OW_O], cb_d[:, 0:OW_O])
            nc.gpsimd.dma_start(bias_sb[:], bias_d)

            def w_qk(k, h):
                o = OW_QK + k * 256 + h * 128
                return cblob[:, o:o + 128]

            def w_v(k):
                o = OW_V + k * 128
                return cblob[:, o:o + 128]

            wo2_sb = cblob[:, OW_O:OW_O + E]
            mask2_sb = cblob[:, OW_M:OW_M + 256].rearrange(
                "p (h c) -> p h c", h=2)
            rk2_sb = cblob[:, OW_R:OW_R + RPC]
            bqk_sb = bias_sb[:, 0:2]
            bvp_sb = bias_sb[:, 2:3]

            # ---- x chunk loads: 0,1 on scalar queue, 2,3 on gpsimd -------
            xtiles = {}
            for pc in range(NPC):
                xtile = xtp.tile([128, KT, PCW], BF16, tag="xt", name="xtile")
                eng = nc.scalar if pc < 3 else nc.gpsimd
                eng.dma_start(xtile[:], xt_d[:, pc, :, :])
                xtiles[pc] = xtile
            nc.gpsimd.dma_start(cblob[:, OW_O:], cb_d[:, OW_O:])

            # ---- PE warm-up bridge while the first DMAs land -------------
            wsc = const.tile([128, 512], BF16)
            nc.vector.memset(wsc[:], 0.0)
            warm = pwo.tile([128, 512], F32, tag="wps", name="warm")
            NWARM = 10
            for i in range(NWARM):
                nc.tensor.matmul(warm[:], wsc[:, 0:128], wsc[:],
                                 start=(i == 0), stop=(i == NWARM - 1))

            # ---- persistent activations ----------------------------------
            # ktv0 = [k_h0 0:64 (DMA shift, also scores lhsT) | v~_h0 64:]
            # ktv1 = [v~_h1 0:64 | k_h1 64:128 (plain copy)]  (vp packed
            #   [v_h1; v_h0] so both stt writes are partition-aligned)
            # kc1  = k_h1 shifted to partitions 0:64 (scores lhsT for h1)
            qkt = [bigp.tile([128, RPC], BF16, name=f"qkt{h}")
                   for h in range(2)]
            ktv0 = bigp.tile([128, RPC], BF16)
            ktv1 = bigp.tile([128, RPC], BF16)
            kc1 = bigp.tile([64, RPC], BF16)
            # row-major chunks, one tile per transpose batch (separate
            # tiles so chunk reads can never alias a later batch's write)
            # kvr*[0] cols = [k 0:64 | v 64:128]; kvr*[1] = [v 0:64 | k 64:]
            kvr_parts = []      # list of (c0, c1, [tile_h0, tile_h1])
            for (c0, c1) in ((0, 8), (8, 12), (12, 16)):
                ts = [bigp.tile([128, c1 - c0, 2 * HD], BF16,
                                name=f"kvr{h}_{c0}") for h in range(2)]
                kvr_parts.append((c0, c1, ts))

            def kvr_at(h, cl):
                for c0, c1, ts in kvr_parts:
                    if c0 <= cl < c1:
                        return ts[h], cl - c0
                raise AssertionError

            ps2t = ps2.tile([128, 4, C], F32, tag="s2", name="s2")
            ppo_t = ppo.tile([128, 4, C], F32, tag="po", name="po")
            pst_t = pstp.tile([HD, 8, HD], F32, tag="st", name="st")
            sp_of = {}          # cl -> [64, 2(head), 64] bf16 state product
            pref = {}           # cl -> [64, 2(head), 64] bf16 S_{<cl}

            def proj(pc):
                xtile = xtiles[pc]
                sl = slice(pc * PCW, (pc + 1) * PCW)
                for h in (0, 1):
                    qk = pjqk.tile([128, PCW], F32, tag="pj", name="qkps")
                    for k in range(KT):
                        nc.tensor.matmul(qk[:], w_qk(k, h), xtile[:, k, :],
                                         start=(k == 0), stop=(k == KT - 1))
                    nc.scalar.activation(qkt[h][:, sl], qk[:], AF.Relu,
                                         bias=bqk_sb[:, h:h + 1])
                # k shifts on sync (scores-critical, FIFO before transposes)
                nc.sync.dma_start(ktv0[0:64, sl], qkt[0][64:128, sl])
                nc.sync.dma_start(kc1[:, sl], qkt[1][64:128, sl])
                # plain-aligned k_h1 copy for the transpose input
                nc.sync.dma_start(ktv1[64:128, sl], qkt[1][64:128, sl])
                # v psum double-buffers by borrowing the (idle until step 1)
                # pwo bank for odd pc — back-to-back projs don't serialize
                # on the DVE stt drain
                pool = pjv if pc % 2 == 0 else pwo
                vp = pool.tile([128, PCW], F32,
                               tag="pjv" if pc % 2 == 0 else "wps",
                               name="vps")
                for k in range(KT):
                    nc.tensor.matmul(vp[:], w_v(k), xtile[:, k, :],
                                     start=(k == 0), stop=(k == KT - 1))
                # v~ = (v + bv) * (1/|k_row|); vp = [v_h1; v_h0] so both
                # halves land partition-aligned
                nc.vector.scalar_tensor_tensor(
                    ktv1[0:64, sl], vp[0:64, :], bvp_sb[0:64, :],
                    rk2_sb[0:64, sl], op0=ALU.add, op1=ALU.mult)
                nc.vector.scalar_tensor_tensor(
                    ktv0[64:128, sl], vp[64:128, :], bvp_sb[64:128, :],
                    rk2_sb[64:128, sl], op0=ALU.add, op1=ALU.mult)

            def transpose_rows(c0, c1):
                # ktv -> row-major kvr for chunks [c0, c1) (one big xbar DMA
                # per head: batching amortizes the ~1us per-op fixed cost)
                sl = slice(c0 * C, c1 * C)
                ts = next(p[2] for p in kvr_parts if p[0] == c0)
                nc.sync.dma_start_transpose(ts[0][:], ktv0[:, sl])
                nc.sync.dma_start_transpose(ts[1][:], ktv1[:, sl])

            # per-head column slices of row-major kvr: [k | v] vs [v | k]
            KSL = (slice(0, HD), slice(HD, 2 * HD))
            VSL = (slice(HD, 2 * HD), slice(0, HD))

            def prework(cl):
                # scores for both heads of chunk cl + causal mask (DVE)
                rows = slice(cl * C, (cl + 1) * C)
                s0 = (cl % 2) * 2
                nc.tensor.matmul(ps2t[:, s0, :], ktv0[0:64, rows],
                                 qkt[0][0:64, rows], start=True, stop=True)
                nc.tensor.matmul(ps2t[:, s0 + 1, :], kc1[:, rows],
                                 qkt[1][0:64, rows], start=True, stop=True)
                at2 = atp.tile([128, 2, C], BF16, name="at2")
                nc.vector.tensor_mul(at2[:], ps2t[:, s0:s0 + 2, :],
                                     mask2_sb[:])
                return at2

            def states_block(cl):
                s = (2 * cl) % 8
                for h in (0, 1):
                    kt, j = kvr_at(h, cl)
                    nc.tensor.matmul(pst_t[:, s + h, :],
                                     kt[:, j, KSL[h]],
                                     kt[:, j, VSL[h]],
                                     start=True, stop=True)
                nxt = cl + 1
                if nxt >= NCH:
                    return
                # prefix fused with the PSUM drain: one DVE op per chunk
                # (pref[cl+1] = pref[cl] + S_cl), no Pool / no extra copy
                pf = ssbp.tile([HD, 2, HD], BF16, tag="pref", bufs=8,
                               name="pref")
                if cl == 0:
                    nc.vector.tensor_copy(pf[:], pst_t[:, s:s + 2, :])
                else:
                    nc.vector.tensor_add(pf[:], pref[cl][:],
                                         pst_t[:, s:s + 2, :])
                pref[nxt] = pf

            def po_block(cl, at2):
                rows = slice(cl * C, (cl + 1) * C)
                s = cl % 4
                for h in (0, 1):
                    kt, j = kvr_at(h, cl)
                    nc.tensor.matmul(ppo_t[h * HD:(h + 1) * HD, s, :],
                                     kt[:, j, VSL[h]], at2[:, h, :],
                                     start=True, stop=(cl == 0))
                    if cl > 0:
                        nc.tensor.matmul(ppo_t[h * HD:(h + 1) * HD, s, :],
                                         pref[cl][:, h, :],
                                         qkt[h][0:64, rows],
                                         start=False, stop=True)
                # po -> SBUF bf16 (ACT; DVE is loaded with mask+states)
                ot = otp.tile([128, C], BF16, name="ot")
                nc.scalar.copy(ot[:], ppo_t[:, s, :])
                return ot

            ob_cur = {}

            def wo_block(cl, ot):
                pw = pwo.tile([128, E], F32, tag="wps", name="wps")
                nc.tensor.matmul(pw[:], ot[:], wo2_sb[:],
                                 start=True, stop=True)
                if cl % 2 == 0:
                    ob_cur["t"] = osbp.tile([128, 2, E], BF16, tag="osb",
                                            name="osb")
                ob = ob_cur["t"]
                j = cl % 2
                # uneven column split: ACT is lighter-loaded than DVE
                nc.scalar.copy(ob[:, j, 0:384], pw[:, 0:384])
                nc.vector.tensor_copy(ob[:, j, 384:E], pw[:, 384:E])
                if j == 1:
                    dst = out_d[(cl - 1) * C:(cl + 1) * C, :].rearrange(
                        "(j p) e -> p j e", j=2)
                    nc.gpsimd.dma_start(dst, ob[:])

            def filler(n):
                # dead N=512 matmuls to keep the PE HAM activity monitor
                # above its throttle threshold (else the clock gate halves
                # the PE clock for the small-matmul attention phase)
                fw = pwo.tile([128, 512], F32, tag="wps", name="fill")
                for i in range(n):
                    nc.tensor.matmul(fw[:], wsc[:, 0:128], wsc[:],
                                     start=(i == 0), stop=(i == n - 1))

            # ---- pipeline ------------------------------------------------
            # Projections are front-loaded (0-2 before the loop, 3 at step
            # 1): the PE streams ~14us of dense N=512 matmuls while the
            # sync-queue shift->transpose convoy resolves, so every
            # row-major kvr tile is ready long before states needs it.
            # step cl: scores(cl+2) | po(cl) | Wo(cl-1) | states(cl+2);
            # the 2-step lookahead keeps the cross-engine consumers (DVE
            # mask, fused prefix-add) off the in-order PE queue's critical
            # path.
            proj(0)
            proj(1)
            transpose_rows(0, 8)
            at_of = {0: prework(0), 1: prework(1)}
            states_block(0)
            states_block(1)
            ot_of = {}
            for cl in range(NCH + 1):
                if cl == 0:
                    proj(2)
                    transpose_rows(8, 12)
                elif cl == 2:
                    proj(3)
                    transpose_rows(12, 16)
                elif cl < NCH:
                    filler(1)
                if cl + 2 <= NCH - 1:
                    at_of[cl + 2] = prework(cl + 2)
                if cl <= NCH - 1:
                    ot_of[cl] = po_block(cl, at_of.pop(cl))
                if cl - 1 >= 0:
                    wo_block(cl - 1, ot_of.pop(cl - 1))
                if cl + 2 <= NCH - 1:
                    states_block(cl + 2)

    nc.compile()
    return nc


def _get_nc():
    if "nc" not in _cache:
        _cache["nc"] = _build()
    return _cache["nc"]


def _host_norms(xs, W, bias):
    """1/max(||relu(xs @ W.T + bias)||, eps) per row, flat [N] f32."""
    p = np.maximum(xs @ W.T + bias, 0.0)
    nrm = np.maximum(np.sqrt(np.sum(p * p, axis=1)), EPS)
    return (1.0 / nrm).astype(np.float32)


def kernel(query, Wq, bq, Wk, bk, Wv, bv, Wo, bo):
    query = np.asarray(query, dtype=np.float32)
    Wq, bq = np.asarray(Wq, np.float32), np.asarray(bq, np.float32)
    Wk, bk = np.asarray(Wk, np.float32), np.asarray(bk, np.float32)
    Wv, bv = np.asarray(Wv, np.float32), np.asarray(bv, np.float32)
    Wo, bo = np.asarray(Wo, np.float32), np.asarray(bo, np.float32)
    assert query.shape == (B, L, E)

    # x = query.reshape(L, B, E) (torch view), then b-major rows
    xs = np.ascontiguousarray(
        query.reshape(L, B, E).transpose(1, 0, 2)).reshape(N, E)

    rq = _host_norms(xs, Wq, bq)
    rk = _host_norms(xs, Wk, bk)

    # per-batch x tiles: [128, pc, kt, n'] with 4KB contiguous rows
    xt_b = []
    rk2_b = []
    for b in range(B):
        xb = xs[b * L:(b + 1) * L]
        xt_b.append(np.ascontiguousarray(
            xb.T.reshape(KT, 128, NPC, PCW).transpose(1, 2, 0, 3)).astype(BF))
        rk2_b.append(np.ascontiguousarray(np.broadcast_to(
            rk[b * L:(b + 1) * L][None, :], (128, RPC))).astype(BF))

    tri = np.triu(np.ones((C, C), np.float32)).astype(BF)
    mask2 = np.ascontiguousarray(
        np.broadcast_to(tri[:, None, :], (C, 2, C))).reshape(C, 2 * C)

    in_maps = []
    for c in range(NCORES):
        b = c // 4
        h0 = 2 * (c % 4)
        cols0 = slice(HD * h0, HD * (h0 + 1))
        cols1 = slice(HD * (h0 + 1), HD * (h0 + 2))
        wqk = np.empty((128, KT, 2, 128), np.float32)
        bqk = np.empty((128, 2), np.float32)
        for h, cols in enumerate((cols0, cols1)):
            wcat = np.concatenate([Wq[cols].T, Wk[cols].T], axis=1)
            wqk[:, :, h, :] = wcat.reshape(KT, 128, 128).transpose(1, 0, 2)
            bqk[:, h] = np.concatenate([bq[cols], bk[cols]])
        # vp psum layout is [v_h1 (0:64) | v_h0 (64:128)] — see ktv comments
        vcat = np.concatenate([Wv[cols1].T, Wv[cols0].T], axis=1)
        wv = vcat.reshape(KT, 128, 128).transpose(1, 0, 2)
        wo2 = np.concatenate([Wo[:, cols0].T, Wo[:, cols1].T], axis=0)
        cb = np.concatenate([
            wqk.reshape(128, KT * 256),
            wv.reshape(128, KT * 128),
            wo2,
            mask2,
            rk2_b[b],
        ], axis=1).astype(BF)
        assert cb.shape == (128, CBLOB_W)
        bias = np.concatenate(
            [bqk, np.concatenate([bv[cols1], bv[cols0]])[:, None]],
            axis=1).astype(np.float32)
        in_maps.append(dict(xt=xt_b[b], cb=cb, bias=bias))

    nc = _get_nc()
    res = bass_utils.run_bass_kernel_spmd(nc, in_maps,
                                          core_ids=list(range(NCORES)))
    total = np.zeros((N, E), np.float32)
    for c in range(NCORES):
        b = c // 4
        total[b * L:(b + 1) * L] += res.results[c]["out"].astype(np.float32)
    total *= rq[:, None]

    out = (total.reshape(B, L, E).transpose(1, 0, 2) + bo).reshape(B, L, E)
    return np.ascontiguousarray(out.astype(np.float32))


# revision 44
# speedup vs baseline: 1.1477x; 1.0980x over previous
"""Bass/Trainium2 kernel v5 for nn_NormAttention (causal linear attention).

Batch+head-sharded SPMD across 8 NeuronCores, no collectives:
core c owns batch b = c//4 and heads {2*(c%4), 2*(c%4)+1} (2048 rows,
2 heads per core: half the x traffic / half the output partial of the
v2 head-only split, and V-proj / Wo run with the full 128-wide array).

Execution shape (per core, rows chunked at C=128, 16 chunks):
  - 4 projection chunks of 512 rows; per chunk 12 N=512 matmuls
    (q|k per head, packed v) — big streaming work for the PE.
  - per row-chunk: 2 score matmuls (N=128, K=64), masked on DVE; po =
    V_rows.T @ at + S_prefix.T @ q (4 matmuls N=128); states K.T@V~
    (2 matmuls N=64); Wo [128,512] (N=512, K=128 both heads packed).
  - row-major K/V tiles come from 6 batched xbar DMA-transposes
    ([128, 512-1024] each; the ~1us/op fixed cost made 16 small ones a
    serial bottleneck in v3).
  - scores and states+prefix run 2 steps ahead of po so the
    states->DVE copy->Pool add->PE inter-matmul cross-engine chain is
    never on the critical path.
  - 2 dead N=512 "filler" matmuls per step keep the PE HAM activity
    window above the clock-gate threshold (otherwise the attention
    phase runs at 1.2GHz instead of 2.4GHz).
  - DMA queues: sync = k shifts + transposes; scalar = x0/x1 loads;
    gpsimd = const blob + x2/x3 + paired output stores.

1/|q| and 1/|k| span all 8 heads (they live on different cores), so
they are computed host-side (as in the graded v2 baseline); 1/|k| is
folded into V on device, 1/|q| + bo into the host-side unshard sum of
the 4 per-batch partials.
"""
import numpy as np
import ml_dtypes

import concourse.bacc as bacc
import concourse.tile as tile
import concourse.mybir as mybir
import concourse.bass_utils as bass_utils

F32 = mybir.dt.float32
BF16 = mybir.dt.bfloat16
BF = ml_dtypes.bfloat16
AF = mybir.ActivationFunctionType
ALU = mybir.AluOpType

B, L, E, H, HD = 2, 2048, 512, 8, 64
N = B * L
NCORES = 8
RPC = 2048              # rows per core (one batch)
KT = 4                  # contraction k-tiles (E // 128)
C = 128                 # attention row-chunk
NCH = RPC // C          # 16 row chunks per core
PCW = 512               # proj chunk width (rows)
NPC = RPC // PCW        # 4 proj chunks
EPS = 1e-12

# const blob column offsets (bf16): wqk | wv | wo2 | mask2 | rk2
OW_QK, OW_V, OW_O, OW_M, OW_R = 0, 1024, 1536, 2048, 2304
CBLOB_W = 2304 + RPC

_cache = {}


def _build():
    nc = bacc.Bacc("TRN2", target_bir_lowering=False, debug=False,
                   num_devices=NCORES)

    xt_d = nc.dram_tensor("xt", [128, NPC, KT, PCW], BF16,
                          kind="ExternalInput").ap()
    cb_d = nc.dram_tensor("cb", [128, CBLOB_W], BF16,
                          kind="ExternalInput").ap()
    bias_d = nc.dram_tensor("bias", [128, 3], F32,
                            kind="ExternalInput").ap()
    out_d = nc.dram_tensor("out", [RPC, E], BF16, kind="ExternalOutput").ap()

    with tile.TileContext(nc) as tc:
        with (
            tc.tile_pool(name="const", bufs=1) as const,
            tc.tile_pool(name="bigp", bufs=1) as bigp,
            tc.tile_pool(name="xtp", bufs=4) as xtp,
            tc.tile_pool(name="atp", bufs=4) as atp,
            tc.tile_pool(name="otp", bufs=4) as otp,
            tc.tile_pool(name="ssbp", bufs=3) as ssbp,
            tc.tile_pool(name="osbp", bufs=3) as osbp,
            tc.tile_pool(name="pjqk", bufs=2, space="PSUM") as pjqk,
            tc.tile_pool(name="pjv", bufs=1, space="PSUM") as pjv,
            tc.tile_pool(name="ps2", bufs=1, space="PSUM") as ps2,
            tc.tile_pool(name="ppo", bufs=1, space="PSUM") as ppo,
            tc.tile_pool(name="pstp", bufs=1, space="PSUM") as pstp,
            tc.tile_pool(name="pwo", bufs=2, space="PSUM") as pwo,
        ):
            # ---- constants: wqk+wv first (proj(0) deps), then the rest
            # (wo2/mask2/rk2 after x3 — they are needed later than x3)
            cblob = const.tile([128, CBLOB_W], BF16)
            bias_sb = const.tile([128, 3], F32)
            nc.gpsimd.dma_start(cblob[:, 0:OW_O], cb_d[:, 0:OW_O])
            nc.gpsimd.dma_start(bias_sb[:], bias_d)

            def w_qk(k, h):
                o = OW_QK + k * 256 + h * 128
                return cblob[:, o:o + 128]

            def w_v(k):
                o = OW_V + k * 128
                return cblob[:, o:o + 128]

            wo2_sb = cblob[:, OW_O:OW_O + E]
            mask2_sb = cblob[:, OW_M:OW_M + 256].rearrange(
                "p (h c) -> p h c", h=2)
            rk2_sb = cblob[:, OW_R:OW_R + RPC]
            bqk_sb = bias_sb[:, 0:2]
            bvp_sb = bias_sb[:, 2:3]

            # ---- x chunk loads: 0,1 on scalar queue, 2,3 on gpsimd -------
            xtiles = {}
            for pc in range(NPC):
                xtile = xtp.tile([128, KT, PCW], BF16, tag="xt", name="xtile")
                eng = nc.scalar if pc < 3 else nc.gpsimd
                eng.dma_start(xtile[:], xt_d[:, pc, :, :])
                xtiles[pc] = xtile
            nc.gpsimd.dma_start(cblob[:, OW_O:], cb_d[:, OW_O:])

            # ---- PE warm-up bridge while the first DMAs land -------------
            wsc = const.tile([128, 512], BF16)
            nc.vector.memset(wsc[:], 0.0)
            warm = pwo.tile([128, 512], F32, tag="wps", name="warm")
            NWARM = 10
            for i in range(NWARM):
                nc.tensor.matmul(warm[:], wsc[:, 0:128], wsc[:],
                                 start=(i == 0), stop=(i == NWARM - 1))

            # ---- persistent activations ----------------------------------
            # ktv0 = [k_h0 0:64 (DMA shift, also scores lhsT) | v~_h0 64:]
            # ktv1 = [v~_h1 0:64 | k_h1 64:128 (plain copy)]  (vp packed
            #   [v_h1; v_h0] so both stt writes are partition-aligned)
            # kc1  = k_h1 shifted to partitions 0:64 (scores lhsT for h1)
            qkt = [bigp.tile([128, RPC], BF16, name=f"qkt{h}")
                   for h in range(2)]
            ktv0 = bigp.tile([128, RPC], BF16)
            ktv1 = bigp.tile([128, RPC], BF16)
            kc1 = bigp.tile([64, RPC], BF16)
            # row-major chunks, one tile per transpose batch (separate
            # tiles so chunk reads can never alias a later batch's write)
            # kvr*[0] cols = [k 0:64 | v 64:128]; kvr*[1] = [v 0:64 | k 64:]
            kvr_parts = []      # list of (c0, c1, [tile_h0, tile_h1])
            for (c0, c1) in ((0, 8), (8, 12), (12, 16)):
                ts = [bigp.tile([128, c1 - c0, 2 * HD], BF16,
                                name=f"kvr{h}_{c0}") for h in range(2)]
                kvr_parts.append((c0, c1, ts))

            def kvr_at(h, cl):
                for c0, c1, ts in kvr_parts:
                    if c0 <= cl < c1:
                        return ts[h], cl - c0
                raise AssertionError

            ps2t = ps2.tile([128, 4, C], F32, tag="s2", name="s2")
            ppo_t = ppo.tile([128, 4, C], F32, tag="po", name="po")
            pst_t = pstp.tile([HD, 8, HD], F32, tag="st", name="st")
            sp_of = {}          # cl -> [64, 2(head), 64] bf16 state product
            pref = {}           # cl -> [64, 2(head), 64] bf16 S_{<cl}

            def proj(pc):
                xtile = xtiles[pc]
                sl = slice(pc * PCW, (pc + 1) * PCW)
                for h in (0, 1):
                    qk = pjqk.tile([128, PCW], F32, tag="pj", name="qkps")
                    for k in range(KT):
                        nc.tensor.matmul(qk[:], w_qk(k, h), xtile[:, k, :],
                                         start=(k == 0), stop=(k == KT - 1))
                    nc.scalar.activation(qkt[h][:, sl], qk[:], AF.Relu,
                                         bias=bqk_sb[:, h:h + 1])
                # the three k copies go to three different queues so the
                # relu -> copies -> transposes chain runs its copies in
                # parallel (the transposes' completion gates every
                # later-emitted attention matmul via shared DMA-sem lanes)
                nc.sync.dma_start(ktv0[0:64, sl], qkt[0][64:128, sl])
                nc.scalar.dma_start(kc1[:, sl], qkt[1][64:128, sl])
                nc.gpsimd.dma_start(ktv1[64:128, sl], qkt[1][64:128, sl])
                # v psum double-buffers by borrowing the (mostly idle) pwo
                # bank for pc1 — back-to-back projs don't serialize on the
                # DVE stt drain
                pool = pwo if pc == 1 else pjv
                vp = pool.tile([128, PCW], F32,
                               tag="wps" if pc == 1 else "pjv",
                               name="vps")
                for k in range(KT):
                    nc.tensor.matmul(vp[:], w_v(k), xtile[:, k, :],
                                     start=(k == 0), stop=(k == KT - 1))
                # v~ = (v + bv) * (1/|k_row|); vp = [v_h1; v_h0] so both
                # halves land partition-aligned
                nc.vector.scalar_tensor_tensor(
                    ktv1[0:64, sl], vp[0:64, :], bvp_sb[0:64, :],
                    rk2_sb[0:64, sl], op0=ALU.add, op1=ALU.mult)
                nc.vector.scalar_tensor_tensor(
                    ktv0[64:128, sl], vp[64:128, :], bvp_sb[64:128, :],
                    rk2_sb[64:128, sl], op0=ALU.add, op1=ALU.mult)

            def transpose_rows(c0, c1):
                # ktv -> row-major kvr for chunks [c0, c1) (one big xbar DMA
                # per head: batching amortizes the ~1us per-op fixed cost)
                sl = slice(c0 * C, c1 * C)
                ts = next(p[2] for p in kvr_parts if p[0] == c0)
                nc.sync.dma_start_transpose(ts[0][:], ktv0[:, sl])
                nc.sync.dma_start_transpose(ts[1][:], ktv1[:, sl])

            # per-head column slices of row-major kvr: [k | v] vs [v | k]
            KSL = (slice(0, HD), slice(HD, 2 * HD))
            VSL = (slice(HD, 2 * HD), slice(0, HD))

            def prework(cl):
                # scores for both heads of chunk cl + causal mask (DVE)
                rows = slice(cl * C, (cl + 1) * C)
                s0 = (cl % 2) * 2
                nc.tensor.matmul(ps2t[:, s0, :], ktv0[0:64, rows],
                                 qkt[0][0:64, rows], start=True, stop=True)
                nc.tensor.matmul(ps2t[:, s0 + 1, :], kc1[:, rows],
                                 qkt[1][0:64, rows], start=True, stop=True)
                at2 = atp.tile([128, 2, C], BF16, name="at2")
                nc.vector.tensor_mul(at2[:], ps2t[:, s0:s0 + 2, :],
                                     mask2_sb[:])
                return at2

            def states_block(cl):
                s = (2 * cl) % 8
                for h in (0, 1):
                    kt, j = kvr_at(h, cl)
                    nc.tensor.matmul(pst_t[:, s + h, :],
                                     kt[:, j, KSL[h]],
                                     kt[:, j, VSL[h]],
                                     start=True, stop=True)
                nxt = cl + 1
                if nxt >= NCH:
                    return
                # prefix fused with the PSUM drain: one DVE op per chunk
                # (pref[cl+1] = pref[cl] + S_cl), no Pool / no extra copy
                pf = ssbp.tile([HD, 2, HD], BF16, tag="pref", bufs=8,
                               name="pref")
                if cl == 0:
                    nc.vector.tensor_copy(pf[:], pst_t[:, s:s + 2, :])
                else:
                    nc.vector.tensor_add(pf[:], pref[cl][:],
                                         pst_t[:, s:s + 2, :])
                pref[nxt] = pf

            def po_block(cl, at2):
                rows = slice(cl * C, (cl + 1) * C)
                s = cl % 4
                for h in (0, 1):
                    kt, j = kvr_at(h, cl)
                    nc.tensor.matmul(ppo_t[h * HD:(h + 1) * HD, s, :],
                                     kt[:, j, VSL[h]], at2[:, h, :],
                                     start=True, stop=(cl == 0))
                    if cl > 0:
                        nc.tensor.matmul(ppo_t[h * HD:(h + 1) * HD, s, :],
                                         pref[cl][:, h, :],
                                         qkt[h][0:64, rows],
                                         start=False, stop=True)
                # po -> SBUF bf16 (ACT; DVE is loaded with mask+states)
                ot = otp.tile([128, C], BF16, name="ot")
                nc.scalar.copy(ot[:], ppo_t[:, s, :])
                return ot

            ob_cur = {}

            def wo_block(cl, ot):
                pw = pwo.tile([128, E], F32, tag="wps", name="wps")
                nc.tensor.matmul(pw[:], ot[:], wo2_sb[:],
                                 start=True, stop=True)
                if cl % 2 == 0:
                    ob_cur["t"] = osbp.tile([128, 2, E], BF16, tag="osb",
                                            name="osb")
                ob = ob_cur["t"]
                j = cl % 2
                # uneven column split: ACT is lighter-loaded than DVE
                nc.scalar.copy(ob[:, j, 0:384], pw[:, 0:384])
                nc.vector.tensor_copy(ob[:, j, 384:E], pw[:, 384:E])
                if j == 1:
                    dst = out_d[(cl - 1) * C:(cl + 1) * C, :].rearrange(
                        "(j p) e -> p j e", j=2)
                    nc.gpsimd.dma_start(dst, ob[:])

            def filler(n):
                # dead N=512 matmuls to keep the PE HAM activity monitor
                # above its throttle threshold (else the clock gate halves
                # the PE clock for the small-matmul attention phase)
                fw = pwo.tile([128, 512], F32, tag="wps", name="fill")
                for i in range(n):
                    nc.tensor.matmul(fw[:], wsc[:, 0:128], wsc[:],
                                     start=(i == 0), stop=(i == n - 1))

            # ---- pipeline ------------------------------------------------
            # Projections are front-loaded (0-2 before the loop, 3 at step
            # 1): the PE streams ~14us of dense N=512 matmuls while the
            # sync-queue shift->transpose convoy resolves, so every
            # row-major kvr tile is ready long before states needs it.
            # step cl: scores(cl+2) | po(cl) | Wo(cl-1) | states(cl+2);
            # the 2-step lookahead keeps the cross-engine consumers (DVE
            # mask, fused prefix-add) off the in-order PE queue's critical
            # path.
            proj(0)
            proj(1)
            transpose_rows(0, 8)
            at_of = {cl: prework(cl) for cl in range(3)}
            states_block(0)
            states_block(1)
            states_block(2)
            ot_of = {}
            # proj(2)/proj(3) are emitted at the END of steps 0/2: the +3
            # lookahead queues ~4us of already-satisfied PE work ahead of
            # them, absorbing the transpose-chain latency that every
            # later-emitted matmul conservatively waits on
            for cl in range(NCH + 1):
                if cl not in (0, 2) and cl < NCH:
                    filler(1)
                if cl <= NCH - 1:
                    ot_of[cl] = po_block(cl, at_of.pop(cl))
                if cl - 1 >= 0:
                    wo_block(cl - 1, ot_of.pop(cl - 1))
                if cl + 3 <= NCH - 1:
                    at_of[cl + 3] = prework(cl + 3)
                    states_block(cl + 3)
                if cl == 0:
                    proj(2)
                    transpose_rows(8, 12)
                elif cl == 2:
                    proj(3)
                    transpose_rows(12, 16)

    nc.compile()
    return nc


def _get_nc():
    if "nc" not in _cache:
        _cache["nc"] = _build()
    return _cache["nc"]


def _host_norms(xs, W, bias):
    """1/max(||relu(xs @ W.T + bias)||, eps) per row, flat [N] f32."""
    p = np.maximum(xs @ W.T + bias, 0.0)
    nrm = np.maximum(np.sqrt(np.sum(p * p, axis=1)), EPS)
    return (1.0 / nrm).astype(np.float32)


def kernel(query, Wq, bq, Wk, bk, Wv, bv, Wo, bo):
    query = np.asarray(query, dtype=np.float32)
    Wq, bq = np.asarray(Wq, np.float32), np.asarray(bq, np.float32)
    Wk, bk = np.asarray(Wk, np.float32), np.asarray(bk, np.float32)
    Wv, bv = np.asarray(Wv, np.float32), np.asarray(bv, np.float32)
    Wo, bo = np.asarray(Wo, np.float32), np.asarray(bo, np.float32)
    assert query.shape == (B, L, E)

    # x = query.reshape(L, B, E) (torch view), then b-major rows
    xs = np.ascontiguousarray(
        query.reshape(L, B, E).transpose(1, 0, 2)).reshape(N, E)

    rq = _host_norms(xs, Wq, bq)
    rk = _host_norms(xs, Wk, bk)

    # per-batch x tiles: [128, pc, kt, n'] with 4KB contiguous rows
    xt_b = []
    rk2_b = []
    for b in range(B):
        xb = xs[b * L:(b + 1) * L]
        xt_b.append(np.ascontiguousarray(
            xb.T.reshape(KT, 128, NPC, PCW).transpose(1, 2, 0, 3)).astype(BF))
        rk2_b.append(np.ascontiguousarray(np.broadcast_to(
            rk[b * L:(b + 1) * L][None, :], (128, RPC))).astype(BF))

    tri = np.triu(np.ones((C, C), np.float32)).astype(BF)
    mask2 = np.ascontiguousarray(
        np.broadcast_to(tri[:, None, :], (C, 2, C))).reshape(C, 2 * C)

    in_maps = []
    for c in range(NCORES):
        b = c // 4
        h0 = 2 * (c % 4)
        cols0 = slice(HD * h0, HD * (h0 + 1))
        cols1 = slice(HD * (h0 + 1), HD * (h0 + 2))
        wqk = np.empty((128, KT, 2, 128), np.float32)
        bqk = np.empty((128, 2), np.float32)
        for h, cols in enumerate((cols0, cols1)):
            wcat = np.concatenate([Wq[cols].T, Wk[cols].T], axis=1)
            wqk[:, :, h, :] = wcat.reshape(KT, 128, 128).transpose(1, 0, 2)
            bqk[:, h] = np.concatenate([bq[cols], bk[cols]])
        # vp psum layout is [v_h1 (0:64) | v_h0 (64:128)] — see ktv comments
        vcat = np.concatenate([Wv[cols1].T, Wv[cols0].T], axis=1)
        wv = vcat.reshape(KT, 128, 128).transpose(1, 0, 2)
        wo2 = np.concatenate([Wo[:, cols0].T, Wo[:, cols1].T], axis=0)
        cb = np.concatenate([
            wqk.reshape(128, KT * 256),
            wv.reshape(128, KT * 128),
            wo2,
            mask2,
            rk2_b[b],
        ], axis=1).astype(BF)
        assert cb.shape == (128, CBLOB_W)
        bias = np.concatenate(
            [bqk, np.concatenate([bv[cols1], bv[cols0]])[:, None]],
            axis=1).astype(np.float32)
        in_maps.append(dict(xt=xt_b[b], cb=cb, bias=bias))

    nc = _get_nc()
    res = bass_utils.run_bass_kernel_spmd(nc, in_maps,
                                          core_ids=list(range(NCORES)))
    total = np.zeros((N, E), np.float32)
    for c in range(NCORES):
        b = c // 4
        total[b * L:(b + 1) * L] += res.results[c]["out"].astype(np.float32)
    total *= rq[:, None]

    out = (total.reshape(B, L, E).transpose(1, 0, 2) + bo).reshape(B, L, E)
    return np.ascontiguousarray(out.astype(np.float32))
